# revision 12
# baseline (speedup 1.0000x reference)
"""Trainium2 Bass kernel: transformer encoder layer (DeepPM style).

B=8 batch elements sharded 1-per-core across 8 NeuronCores.
Per core everything is computed feature-major ("T layout": [d, token])
so no activation transposes are needed until the very end:

  - QKV proj:   lhsT = W.T (host-transposed), rhs = x.T
  - scores.T[k,q] per head via PE (K=32 contraction)
  - softmax without max-subtraction: exp on ACT, additive mask folded in
    multiplicatively (attn = exp(scale*qk) * E, E = exp(mask) host-built)
  - denominator via ones-column appended to V (row 32 of ctx psum)
  - ctx.T = V_aug.T @ attn  (lhsT = token-major V tile)
  - per-head normalize: reciprocal + selector-matmul broadcast
  - out/proj collapsed into one matmul (Wc = proj_w @ out_w, host-folded
    biases), fp32 residual, FFN with fused gelu+bias on ACT
  - final PE transpose to token-major with padded-row zeroing fused into
    the psum->sbuf copy (scale = 0/1 per-partition mask)
"""

import numpy as np
import ml_dtypes
from contextlib import ExitStack

BF16 = ml_dtypes.bfloat16
F32 = np.float32

B, L, D, H, DFF = 8, 1024, 256, 8, 2048
DH = D // H          # 32
P = 128
NKT = L // P         # 8 token tiles
NDT = D // P         # 2 feature tiles
NF1 = DFF // P       # 16
QCW = 512            # q-chunk width (max moving free dim)
NQC = L // QCW       # 2
NCORES = 8

_BUILT = {}


def _build_module(n_iters: int = 1):
    import concourse.tile as tile
    import concourse.mybir as mybir
    from concourse import bacc
    from concourse.masks import make_identity

    dt = mybir.dt
    AF = mybir.ActivationFunctionType
    OP = mybir.AluOpType

    nc = bacc.Bacc("TRN2", target_bir_lowering=False, debug=False)

    def din(name, shape, dtype):
        return nc.dram_tensor(name, shape, dtype, kind="ExternalInput").ap()

    xtb = din("xtb", [P, NDT, L], dt.bfloat16)
    xt32 = din("xt32", [P, NDT, L], dt.float32)
    et = din("et", [P, NKT, L], dt.bfloat16)
    wqk = din("wqk", [P, NDT, 2 * D], dt.bfloat16)
    wv = din("wv", [P, NDT, D], dt.bfloat16)
    wc = din("wc", [P, NDT, D], dt.bfloat16)
    wf1 = din("wf1", [P, NDT, DFF], dt.bfloat16)
    wf2 = din("wf2", [P, NF1, D], dt.bfloat16)
    bqk = din("bqk", [P, 4], dt.float32)
    bc = din("bc", [P, NDT], dt.float32)
    bf1 = din("bf1", [P, NF1], dt.float32)
    bf2 = din("bf2", [P, NDT], dt.float32)
    sel = din("sel", [64, P], dt.bfloat16)
    qm = din("qm", [P, NKT], dt.float32)
    y = nc.dram_tensor("y", [L, D], dt.float32, kind="ExternalOutput").ap()

    with tile.TileContext(nc) as tc, ExitStack() as ctx:
        consts = ctx.enter_context(tc.tile_pool(name="consts", bufs=1))
        acts = ctx.enter_context(tc.tile_pool(name="acts", bufs=1))
        attnp = ctx.enter_context(tc.tile_pool(name="attnp", bufs=2))
        outp = ctx.enter_context(tc.tile_pool(name="outp", bufs=3))
        psum = ctx.enter_context(tc.tile_pool(name="ps", bufs=2, space="PSUM"))
        psS = ctx.enter_context(tc.tile_pool(name="psS", bufs=3, space="PSUM"))
        psC = ctx.enter_context(tc.tile_pool(name="psC", bufs=2, space="PSUM"))

        # ---- constants (loaded once, reused across iterations) ----
        c_id32 = consts.tile([P, P], dt.float32, tag="id32")
        make_identity(nc, c_id32)
        c_sel = consts.tile([64, P], dt.bfloat16, tag="sel")
        nc.sync.dma_start(out=c_sel, in_=sel)
        c_wqk = consts.tile([P, NDT, 2 * D], dt.bfloat16, tag="wqk")
        nc.sync.dma_start(out=c_wqk, in_=wqk)
        c_wv = consts.tile([P, NDT, D], dt.bfloat16, tag="wv")
        nc.sync.dma_start(out=c_wv, in_=wv)
        c_wc = consts.tile([P, NDT, D], dt.bfloat16, tag="wc")
        nc.sync.dma_start(out=c_wc, in_=wc)
        c_wf1 = consts.tile([P, NDT, DFF], dt.bfloat16, tag="wf1")
        nc.sync.dma_start(out=c_wf1, in_=wf1)
        c_wf2 = consts.tile([P, NF1, D], dt.bfloat16, tag="wf2")
        nc.sync.dma_start(out=c_wf2, in_=wf2)
        c_bqk = consts.tile([P, 4], dt.float32, tag="bqk")
        nc.sync.dma_start(out=c_bqk, in_=bqk)
        c_bc = consts.tile([P, NDT], dt.float32, tag="bc")
        nc.sync.dma_start(out=c_bc, in_=bc)
        c_bf1 = consts.tile([P, NF1], dt.float32, tag="bf1")
        nc.sync.dma_start(out=c_bf1, in_=bf1)
        c_bf2 = consts.tile([P, NDT], dt.float32, tag="bf2")
        nc.sync.dma_start(out=c_bf2, in_=bf2)
        c_qm = consts.tile([P, NKT], dt.float32, tag="qm")
        nc.sync.dma_start(out=c_qm, in_=qm)

        for _ in range(n_iters):
            c_xtb = acts.tile([P, NDT, L], dt.bfloat16, tag="xtb")
            nc.sync.dma_start(out=c_xtb, in_=xtb)
            c_x32 = acts.tile([P, NDT, L], dt.float32, tag="x32")
            nc.sync.dma_start(out=c_x32, in_=xt32)
            c_e = acts.tile([P, NKT, L], dt.bfloat16, tag="et")
            nc.sync.dma_start(out=c_e, in_=et)

            # ---- Q,K projections (feature-major, scale folded into Q) ----
            # Head h lives at partition base (h%2)*32, free index h//2 so
            # every per-head matmul operand starts at partition 0 or 32.
            q_sb = acts.tile([2 * DH, 4, L], dt.bfloat16, tag="q")
            k_sb = acts.tile([2 * DH, 4, L], dt.bfloat16, tag="k")
            for mt in range(4):
                dst = q_sb if mt < 2 else k_sb
                for qc in range(NQC):
                    qs = slice(qc * QCW, (qc + 1) * QCW)
                    ps = psum.tile([P, QCW], dt.float32, tag="mm")
                    for kt in range(NDT):
                        nc.tensor.matmul(
                            ps,
                            lhsT=c_wqk[:, kt, mt * P:(mt + 1) * P],
                            rhs=c_xtb[:, kt, qs],
                            start=(kt == 0),
                            stop=(kt == NDT - 1),
                        )
                    for i in range(4):
                        h = (mt % 2) * 4 + i
                        nc.scalar.activation(
                            out=dst[(h % 2) * DH:(h % 2) * DH + DH, h // 2, qs],
                            in_=ps[i * DH:(i + 1) * DH, :],
                            func=AF.Identity,
                            bias=c_bqk[i * DH:(i + 1) * DH, mt:mt + 1],
                        )

            # ---- V (token-major) + ones column for the denominator ----
            vaug = acts.tile([P, NKT, H, DH + 1], dt.bfloat16, tag="vaug")
            nc.vector.memset(vaug, 1.0)
            for tt in range(NKT):
                ps = psum.tile([P, D], dt.float32, tag="mm")
                for kt in range(NDT):
                    nc.tensor.matmul(
                        ps,
                        lhsT=c_xtb[:, kt, tt * P:(tt + 1) * P],
                        rhs=c_wv[:, kt, :],
                        start=(kt == 0),
                        stop=(kt == NDT - 1),
                    )
                nc.scalar.activation(
                    out=vaug[:, tt, :, 0:DH],
                    in_=ps.rearrange("p (h d) -> p h d", h=H),
                    func=AF.Copy,
                )

            # ---- attention ----
            ctxu = acts.tile([P, NDT, L], dt.bfloat16, tag="ctxu")
            # den row for head h (within its q-chunk tile): (h//4)*32 + h%4,
            # so the 4 heads of feature-tile dvt sit contiguously at base
            # dvt*32 (a legal matmul base partition).
            den_flat = acts.tile([1, NQC, H, QCW], dt.float32, tag="denf")
            for h in range(H):
                mtq = h // 4
                po = (h % 4) * DH
                hb_ = (h % 2) * DH
                hf = h // 2
                at = attnp.tile([P, NKT, L], dt.bfloat16, tag="attn")
                for kt in range(NKT):
                    for qc in range(NQC):
                        qs = slice(qc * QCW, (qc + 1) * QCW)
                        ps = psS.tile([P, QCW], dt.float32, tag="score")
                        nc.tensor.matmul(
                            ps,
                            lhsT=k_sb[hb_:hb_ + DH, hf, kt * P:(kt + 1) * P],
                            rhs=q_sb[hb_:hb_ + DH, hf, qs],
                            start=True, stop=True,
                        )
                        nc.scalar.activation(out=at[:, kt, qs], in_=ps, func=AF.Exp)
                        nc.vector.tensor_mul(
                            out=at[:, kt, qs], in0=at[:, kt, qs], in1=c_e[:, kt, qs]
                        )
                for qc in range(NQC):
                    qs = slice(qc * QCW, (qc + 1) * QCW)
                    pc = psC.tile([DH + 1, QCW], dt.float32, tag="ctx")
                    for kt in range(NKT):
                        nc.tensor.matmul(
                            pc,
                            lhsT=vaug[:, kt, h, :],
                            rhs=at[:, kt, qs],
                            start=(kt == 0),
                            stop=(kt == NKT - 1),
                        )
                    nc.scalar.activation(
                        out=ctxu[po:po + DH, mtq, qs], in_=pc[0:DH, :], func=AF.Copy
                    )
                    nc.vector.tensor_copy(
                        out=den_flat[0:1, qc, h, :], in_=pc[DH:DH + 1, :]
                    )

            # ---- normalize: reciprocal + selector-broadcast ----
            # Scatter den rows to partition bases {0,32}+{0..3} so the
            # selector matmul rhs slices start at legal base partitions.
            dens = [acts.tile([64, QCW], dt.float32, tag=f"den{qc}",
                              name=f"den{qc}") for qc in range(NQC)]
            recs = [acts.tile([64, QCW], dt.float32, tag=f"rec{qc}",
                              name=f"rec{qc}") for qc in range(NQC)]
            recbs = [acts.tile([64, QCW], dt.bfloat16, tag=f"recb{qc}",
                               name=f"recb{qc}") for qc in range(NQC)]
            for qc in range(NQC):
                for dvt in range(NDT):
                    nc.sync.dma_start(
                        out=dens[qc][dvt * 32:dvt * 32 + 4, :],
                        in_=den_flat[0:1, qc, 4 * dvt:4 * dvt + 4, :],
                    )
                nc.vector.reciprocal(out=recs[qc], in_=dens[qc])
                nc.gpsimd.tensor_copy(out=recbs[qc], in_=recs[qc])
            ctxn = acts.tile([P, NDT, L], dt.bfloat16, tag="ctxn")
            for dvt in range(NDT):
                for qc in range(NQC):
                    qs = slice(qc * QCW, (qc + 1) * QCW)
                    pr = psum.tile([P, QCW], dt.float32, tag="mm")
                    nc.tensor.matmul(
                        pr,
                        lhsT=c_sel[dvt * 32:dvt * 32 + 4, :],
                        rhs=recbs[qc][dvt * 32:dvt * 32 + 4, :],
                        start=True, stop=True,
                    )
                    nc.vector.tensor_mul(
                        out=ctxn[:, dvt, qs], in0=ctxu[:, dvt, qs], in1=pr
                    )

            # ---- combined out+proj matmul, fp32 residual ----
            h32 = acts.tile([P, NDT, L], dt.float32, tag="h32")
            hb = acts.tile([P, NDT, L], dt.bfloat16, tag="hb")
            for mt in range(NDT):
                for qc in range(NQC):
                    qs = slice(qc * QCW, (qc + 1) * QCW)
                    ps = psum.tile([P, QCW], dt.float32, tag="mm")
                    for kt in range(NDT):
                        nc.tensor.matmul(
                            ps,
                            lhsT=c_wc[:, kt, mt * P:(mt + 1) * P],
                            rhs=ctxn[:, kt, qs],
                            start=(kt == 0),
                            stop=(kt == NDT - 1),
                        )
                    nc.vector.scalar_tensor_tensor(
                        out=h32[:, mt, qs], in0=ps, scalar=c_bc[:, mt:mt + 1],
                        in1=c_x32[:, mt, qs], op0=OP.add, op1=OP.add,
                    )
                    nc.gpsimd.tensor_copy(out=hb[:, mt, qs], in_=h32[:, mt, qs])

            # ---- FFN ----
            g = acts.tile([P, NF1, L], dt.bfloat16, tag="g")
            for mt in range(NF1):
                for qc in range(NQC):
                    qs = slice(qc * QCW, (qc + 1) * QCW)
                    ps = psS.tile([P, QCW], dt.float32, tag="score")
                    for kt in range(NDT):
                        nc.tensor.matmul(
                            ps,
                            lhsT=c_wf1[:, kt, mt * P:(mt + 1) * P],
                            rhs=hb[:, kt, qs],
                            start=(kt == 0),
                            stop=(kt == NDT - 1),
                        )
                    nc.scalar.activation(
                        out=g[:, mt, qs], in_=ps,
                        func=AF.Gelu, bias=c_bf1[:, mt:mt + 1],
                    )
            o32 = acts.tile([P, NDT, L], dt.float32, tag="o32")
            for mt in range(NDT):
                for qc in range(NQC):
                    qs = slice(qc * QCW, (qc + 1) * QCW)
                    ps = psS.tile([P, QCW], dt.float32, tag="score")
                    for kt in range(NF1):
                        nc.tensor.matmul(
                            ps,
                            lhsT=c_wf2[:, kt, mt * P:(mt + 1) * P],
                            rhs=g[:, kt, qs],
                            start=(kt == 0),
                            stop=(kt == NF1 - 1),
                        )
                    nc.vector.scalar_tensor_tensor(
                        out=o32[:, mt, qs], in0=ps, scalar=c_bf2[:, mt:mt + 1],
                        in1=h32[:, mt, qs], op0=OP.add, op1=OP.add,
                    )

            # ---- transpose to token-major, zero padded rows, store ----
            for tt in range(NKT):
                ot = outp.tile([P, D], dt.float32, tag="ot")
                for dtt in range(NDT):
                    pt = psum.tile([P, P], dt.float32, tag="mm")
                    nc.tensor.transpose(
                        pt, o32[:, dtt, tt * P:(tt + 1) * P], c_id32
                    )
                    nc.scalar.activation(
                        out=ot[:, dtt * P:(dtt + 1) * P], in_=pt,
                        func=AF.Copy, scale=c_qm[:, tt:tt + 1],
                    )
                nc.sync.dma_start(out=y[tt * P:(tt + 1) * P, :], in_=ot)

    nc.compile()
    return nc


def _get_module(n_iters: int = 1):
    if n_iters not in _BUILT:
        _BUILT[n_iters] = _build_module(n_iters)
    return _BUILT[n_iters]


def _rearr(a, nt):
    """[nt*128, F] row-major -> device layout [128, nt, F]."""
    f = a.shape[1]
    return np.ascontiguousarray(a.reshape(nt, P, f).transpose(1, 0, 2))


def prepare_in_maps(inputs):
    src = np.asarray(inputs["src"], F32)
    mask = np.asarray(inputs["src_key_padding_mask"])
    in_proj_w = np.asarray(inputs["in_proj_w"], F32)
    in_proj_b = np.asarray(inputs["in_proj_b"], F32)
    out_w = np.asarray(inputs["out_w"], F32)
    out_b = np.asarray(inputs["out_b"], F32)
    proj_w = np.asarray(inputs["proj_w"], F32)
    proj_b = np.asarray(inputs["proj_b"], F32)
    ff1_w = np.asarray(inputs["ff1_w"], F32)
    ff1_b = np.asarray(inputs["ff1_b"], F32)
    ff2_w = np.asarray(inputs["ff2_w"], F32)
    ff2_b = np.asarray(inputs["ff2_b"], F32)

    scale = 1.0 / np.sqrt(F32(DH))
    wq = in_proj_w[:D] * scale
    bq = in_proj_b[:D] * scale
    wk = in_proj_w[D:2 * D]
    bk = in_proj_b[D:2 * D]
    wv_ = in_proj_w[2 * D:]
    bv = in_proj_b[2 * D:]

    wqk_dev = _rearr(np.concatenate([wq, wk], 0).T, NDT).astype(BF16)
    wv_dev = _rearr(wv_.T, NDT).astype(BF16)
    wc_mat = proj_w @ out_w
    wc_dev = _rearr(wc_mat.T, NDT).astype(BF16)
    bo2 = out_b + out_w @ bv
    bc_vec = proj_w @ bo2 + proj_b
    wf1_dev = _rearr(ff1_w.T, NDT).astype(BF16)
    wf2_dev = _rearr(ff2_w.T, NF1).astype(BF16)

    bqk_dev = np.ascontiguousarray(
        np.concatenate([bq, bk]).reshape(4, P).T).astype(F32)
    bc_dev = np.ascontiguousarray(bc_vec.reshape(NDT, P).T).astype(F32)
    bf1_dev = np.ascontiguousarray(ff1_b.reshape(NF1, P).T).astype(F32)
    bf2_dev = np.ascontiguousarray(ff2_b.reshape(NDT, P).T).astype(F32)

    sel_dev = np.zeros((64, P), BF16)
    for j in range(4):
        sel_dev[j, j * DH:(j + 1) * DH] = 1
        sel_dev[32 + j, j * DH:(j + 1) * DH] = 1

    shared = {
        "wqk": wqk_dev, "wv": wv_dev, "wc": wc_dev,
        "wf1": wf1_dev, "wf2": wf2_dev,
        "bqk": bqk_dev, "bc": bc_dev, "bf1": bf1_dev, "bf2": bf2_dev,
        "sel": sel_dev,
    }

    ki = np.arange(L, dtype=F32)[:, None]
    qi = np.arange(L, dtype=F32)[None, :]
    dist = np.abs(qi - ki)

    in_maps = []
    for b in range(NCORES):
        s = int((~mask[b]).sum())
        xT = src[b].T  # [D, L]
        m = (s - dist) / F32(s)
        e = np.exp(m).astype(F32) * (np.arange(L)[:, None] < s)
        im = dict(shared)
        im["xtb"] = _rearr(xT, NDT).astype(BF16)
        im["xt32"] = _rearr(xT, NDT).astype(F32)
        im["et"] = _rearr(e.astype(F32), NKT).astype(BF16)
        im["qm"] = np.ascontiguousarray(
            (np.arange(L) < s).astype(F32).reshape(NKT, P).T)
        in_maps.append(im)
    return in_maps


def run_on_device(inputs, n_iters: int = 1, trace: bool = False):
    from concourse import bass_utils
    nc = _get_module(n_iters)
    in_maps = prepare_in_maps(inputs)
    res = bass_utils.run_bass_kernel_spmd(
        nc, in_maps, core_ids=list(range(NCORES)), trace=trace)
    return res


def kernel(**inputs) -> np.ndarray:
    res = run_on_device(inputs)
    out = np.stack([res.results[b]["y"] for b in range(NCORES)], axis=0)
    return out.astype(F32)


# revision 45
# speedup vs baseline: 1503.1134x; 1503.1134x over previous
"""Trainium2 Bass kernel: transformer encoder layer (DeepPM style).

B=8 batch elements sharded 1-per-core across 8 NeuronCores.
Per core everything is computed feature-major ("T layout": [d, token])
so no activation transposes are needed until the very end:

  - QKV proj:   lhsT = W.T (host-transposed), rhs = x.T
  - scores.T[k,q] per head via PE (K=32 contraction)
  - softmax without max-subtraction: exp on ACT, additive mask folded in
    multiplicatively (attn = exp(scale*qk) * E, E = exp(mask) host-built)
  - denominator via ones-column appended to V (row 32 of ctx psum)
  - ctx.T = V_aug.T @ attn  (lhsT = token-major V tile)
  - per-head normalize: reciprocal + selector-matmul broadcast
  - out/proj collapsed into one matmul (Wc = proj_w @ out_w, host-folded
    biases), fp32 residual, FFN with fused gelu+bias on ACT
  - final PE transpose to token-major with padded-row zeroing fused into
    the psum->sbuf copy (scale = 0/1 per-partition mask)
"""

import numpy as np
import ml_dtypes
from contextlib import ExitStack

BF16 = ml_dtypes.bfloat16
F32 = np.float32

B, L, D, H, DFF = 8, 1024, 256, 8, 2048
DH = D // H          # 32
P = 128
NKT = L // P         # 8 token tiles
NDT = D // P         # 2 feature tiles
NF1 = DFF // P       # 16
QCW = 512            # q-chunk width (max moving free dim)
NQC = L // QCW       # 2
NCORES = 8

_BUILT = {}


def _build_module(n_iters: int = 1):
    import concourse.tile as tile
    import concourse.mybir as mybir
    from concourse import bacc
    from concourse.masks import make_identity

    dt = mybir.dt
    AF = mybir.ActivationFunctionType
    OP = mybir.AluOpType

    nc = bacc.Bacc("TRN2", target_bir_lowering=False, debug=False)

    def din(name, shape, dtype):
        return nc.dram_tensor(name, shape, dtype, kind="ExternalInput").ap()

    xtb = din("xtb", [P, NDT, L], dt.bfloat16)
    xt32 = din("xt32", [P, NDT, L], dt.float32)
    ed = din("ed", [P, NKT, QCW], dt.bfloat16)
    qauxp = din("qauxp", [2, 3, 4, L], dt.bfloat16)
    qauxm = din("qauxm", [2, 3, 4, L], dt.bfloat16)
    kaux = din("kaux", [2, 3, 4, L], dt.bfloat16)
    wqk = din("wqk", [P, NDT, 2 * D], dt.bfloat16)
    wv = din("wv", [P, NDT, D], dt.bfloat16)
    wc = din("wc", [P, NDT, D], dt.bfloat16)
    wf1 = din("wf1", [P, NDT, DFF], dt.bfloat16)
    wf2 = din("wf2", [P, NF1, D], dt.bfloat16)
    bqk = din("bqk", [P, 4], dt.float32)
    bc = din("bc", [P, NDT], dt.float32)
    bf1 = din("bf1", [P, NF1], dt.float32)
    bf2 = din("bf2", [P, NDT], dt.float32)
    sel = din("sel", [64, P], dt.bfloat16)
    qm = din("qm", [P, NKT], dt.float32)
    y = nc.dram_tensor("y", [L, D], dt.float32, kind="ExternalOutput").ap()

    with tile.TileContext(nc) as tc, ExitStack() as ctx:
        consts = ctx.enter_context(tc.tile_pool(name="consts", bufs=1))
        acts = ctx.enter_context(tc.tile_pool(name="acts", bufs=1))
        attnp = ctx.enter_context(tc.tile_pool(name="attnp", bufs=4))
        outp = ctx.enter_context(tc.tile_pool(name="outp", bufs=3))
        psum = ctx.enter_context(tc.tile_pool(name="ps", bufs=2, space="PSUM"))
        psS = ctx.enter_context(tc.tile_pool(name="psS", bufs=2, space="PSUM"))
        psC = ctx.enter_context(tc.tile_pool(name="psC", bufs=2, space="PSUM"))

        # ---- constants; critical-path loads first, bulk weights on SWDGE ----
        c_wqk = consts.tile([P, NDT, 2 * D], dt.bfloat16, tag="wqk")
        nc.sync.dma_start(out=c_wqk, in_=wqk)
        c_bqk = consts.tile([P, 4], dt.float32, tag="bqk")
        nc.sync.dma_start(out=c_bqk, in_=bqk)
        c_wv = consts.tile([P, NDT, D], dt.bfloat16, tag="wv")
        c_sel = consts.tile([64, P], dt.bfloat16, tag="sel")
        c_qm = consts.tile([P, NKT], dt.float32, tag="qm")
        c_id32 = consts.tile([P, P], dt.float32, tag="id32")
        make_identity(nc, c_id32)
        c_wc = consts.tile([P, NDT, D], dt.bfloat16, tag="wc")
        c_bc = consts.tile([P, NDT], dt.float32, tag="bc")
        c_wf1 = consts.tile([P, NDT, DFF], dt.bfloat16, tag="wf1")
        c_bf1 = consts.tile([P, NF1], dt.float32, tag="bf1")
        c_wf2 = consts.tile([P, NF1, D], dt.bfloat16, tag="wf2")
        c_bf2 = consts.tile([P, NDT], dt.float32, tag="bf2")

        for it_ in range(n_iters):
            c_xtb = acts.tile([P, NDT, L], dt.bfloat16, tag="xtb")
            nc.sync.dma_start(out=c_xtb, in_=xtb)
            if it_ == 0:
                nc.sync.dma_start(out=c_wv, in_=wv)
            q_p = acts.tile([P, 4, L], dt.bfloat16, tag="qp")
            q_m = acts.tile([P, 4, L], dt.bfloat16, tag="qm_")
            k2 = acts.tile([P, 4, L], dt.bfloat16, tag="k2")
            for r_ in range(3):
                nc.sync.dma_start(
                    out=q_p.rearrange("(g r) t q -> g r t q", r=64)[:, 32 + r_, :, :],
                    in_=qauxp[:, r_, :, :],
                )
                nc.sync.dma_start(
                    out=q_m.rearrange("(g r) t q -> g r t q", r=64)[:, 32 + r_, :, :],
                    in_=qauxm[:, r_, :, :],
                )
                nc.sync.dma_start(
                    out=k2.rearrange("(g r) t q -> g r t q", r=64)[:, 32 + r_, :, :],
                    in_=kaux[:, r_, :, :],
                )
            c_ed = acts.tile([P, NKT, QCW], dt.bfloat16, tag="ed")
            for kt in range(NKT // 2):
                nc.sync.dma_start(out=c_ed[:, kt, :], in_=ed[:, kt, :])
            c_x32 = acts.tile([P, NDT, L], dt.float32, tag="x32")
            nc.sync.dma_start(out=c_x32, in_=xt32)
            for kt in range(NKT // 2, NKT):
                nc.sync.dma_start(out=c_ed[:, kt, :], in_=ed[:, kt, :])

            # ---- Q,K projections (feature-major, scale folded into Q) ----
            # Head h at partition base (h%2)*64, free index h//2; row base+32
            # holds the aux contraction row for the separable-mask trick:
            # K aux = +/-1, Q aux = -q/s, so a K=33 matmul adds -+q/s to the
            # scores while the per-k +-k/s rides in the exp bias.
            for mt in (0, 2, 1, 3):
                dst = q_p if mt < 2 else k2
                for qc in range(NQC):
                    qs = slice(qc * QCW, (qc + 1) * QCW)
                    ps = psum.tile([P, QCW], dt.float32, tag="mm")
                    for kt in range(NDT):
                        nc.tensor.matmul(
                            ps,
                            lhsT=c_wqk[:, kt, mt * P:(mt + 1) * P],
                            rhs=c_xtb[:, kt, qs],
                            start=(kt == 0),
                            stop=(kt == NDT - 1),
                        )
                    for i in range(4):
                        h = (mt % 2) * 4 + i
                        d_ = dst[(h % 2) * 64:(h % 2) * 64 + DH, h // 2, qs]
                        s_ = ps[i * DH:(i + 1) * DH, :]
                        b_ = c_bqk[i * DH:(i + 1) * DH, mt:mt + 1]
                        nc.vector.tensor_scalar_add(out=d_, in0=s_, scalar1=b_)

            for hf in range(4):
                for g in range(2):
                    nc.gpsimd.tensor_copy(
                        out=q_m[g * 64:g * 64 + DH, hf, :],
                        in_=q_p[g * 64:g * 64 + DH, hf, :],
                    )

            # ---- V (token-major) + ones column for the denominator ----
            vaug = acts.tile([P, NKT, H, DH + 1], dt.bfloat16, tag="vaug")
            nc.vector.memset(vaug[:, :, :, DH:DH + 1], 1.0)
            for tt in range(NKT):
                ps = psum.tile([P, D], dt.float32, tag="mm")
                for kt in range(NDT):
                    nc.tensor.matmul(
                        ps,
                        lhsT=c_xtb[:, kt, tt * P:(tt + 1) * P],
                        rhs=c_wv[:, kt, :],
                        start=(kt == 0),
                        stop=(kt == NDT - 1),
                    )
                nc.scalar.activation(
                    out=vaug[:, tt, :, 0:DH],
                    in_=ps.rearrange("p (h d) -> p h d", h=H),
                    func=AF.Copy,
                )

            if it_ == 0:
                nc.sync.dma_start(out=c_sel, in_=sel)
                nc.sync.dma_start(out=c_qm, in_=qm)
                nc.sync.dma_start(out=c_wc, in_=wc)
                nc.sync.dma_start(out=c_bc, in_=bc)
                nc.sync.dma_start(out=c_wf1, in_=wf1)
                nc.sync.dma_start(out=c_bf1, in_=bf1)
                nc.sync.dma_start(out=c_wf2, in_=wf2)
                nc.sync.dma_start(out=c_bf2, in_=bf2)

            # ---- attention, q-chunk major; per-chunk full tail ----
            from concourse.tile import add_dep_helper
            den_flat = acts.tile([1, NQC, H, QCW], dt.float32, tag="denf")
            exp_by = {}
            h32s, hbs = [], []
            for qc in range(NQC):
                qs = slice(qc * QCW, (qc + 1) * QCW)
                ctxu = acts.tile([P, NDT, QCW], dt.bfloat16, tag=f"ctxu{qc}",
                                 name=f"ctxu{qc}")
                for h in range(H):
                    mtq = h // 4
                    po = (h % 4) * DH
                    hb_ = (h % 2) * 64
                    hf = h // 2
                    at = attnp.tile([P, NKT, QCW], dt.bfloat16, tag="attn")
                    pr_order = [p_ for p_ in range(4) if p_ // 2 <= qc] + \
                               [p_ for p_ in range(4) if p_ // 2 > qc]
                    for pr in pr_order:
                        ps = psS.tile([P, 2 * QCW], dt.float32, tag="score")
                        diag = (pr // 2 == qc)
                        for j in range(2):
                            kt = 2 * pr + j
                            half = ps[:, j * QCW:(j + 1) * QCW]
                            if diag:
                                nc.tensor.matmul(
                                    half,
                                    lhsT=k2[hb_:hb_ + DH, hf, kt * P:(kt + 1) * P],
                                    rhs=q_p[hb_:hb_ + DH, hf, qs],
                                    start=True, stop=True,
                                )
                            else:
                                qv = q_p if kt < 4 * qc else q_m
                                nc.tensor.matmul(
                                    half,
                                    lhsT=k2[hb_:hb_ + DH + 3, hf,
                                            kt * P:(kt + 1) * P],
                                    rhs=qv[hb_:hb_ + DH + 3, hf, qs],
                                    start=True, stop=True,
                                )
                        dst2 = at[:, 2 * pr:2 * pr + 2, :].rearrange(
                            "p a b -> p (a b)")
                        ei = nc.scalar.activation(out=dst2, in_=ps,
                                                  func=AF.Exp).ins
                        exp_by.setdefault((qc, h), []).append(ei)
                        if diag:
                            nc.vector.tensor_mul(
                                out=dst2, in0=dst2,
                                in1=c_ed[:, 2 * pr:2 * pr + 2, :].rearrange(
                                    "p a b -> p (a b)"),
                            )
                    pc = psC.tile([DH + 1, QCW], dt.float32, tag="ctx")
                    for kt in range(NKT):
                        nc.tensor.matmul(
                            pc,
                            lhsT=vaug[:, kt, h, :],
                            rhs=at[:, kt, :],
                            start=(kt == 0),
                            stop=(kt == NKT - 1),
                        )
                    nc.vector.tensor_copy(
                        out=ctxu[po:po + DH, mtq, :], in_=pc[0:DH, :]
                    )
                    nc.vector.tensor_copy(
                        out=den_flat[0:1, qc, h, :], in_=pc[DH:DH + 1, :]
                    )

                # -- normalize (DVE/PE/Pool only) --
                den = acts.tile([64, QCW], dt.float32, tag=f"den{qc}",
                                name=f"den{qc}")
                rec = acts.tile([64, QCW], dt.float32, tag=f"rec{qc}",
                                name=f"rec{qc}")
                recb = acts.tile([64, QCW], dt.bfloat16, tag=f"recb{qc}",
                                 name=f"recb{qc}")
                for dvt in range(NDT):
                    nc.sync.dma_start(
                        out=den[dvt * 32:dvt * 32 + 4, :],
                        in_=den_flat[0:1, qc, 4 * dvt:4 * dvt + 4, :],
                    )
                nc.vector.reciprocal(out=rec, in_=den)
                nc.gpsimd.tensor_copy(out=recb, in_=rec)
                ctxn = acts.tile([P, NDT, QCW], dt.bfloat16, tag=f"ctxn{qc}",
                                 name=f"ctxn{qc}")
                for dvt in range(NDT):
                    pr_ = psum.tile([P, QCW], dt.float32, tag="mm")
                    nc.tensor.matmul(
                        pr_,
                        lhsT=c_sel[dvt * 32:dvt * 32 + 4, :],
                        rhs=recb[dvt * 32:dvt * 32 + 4, :],
                        start=True, stop=True,
                    )
                    nc.vector.tensor_mul(
                        out=ctxn[:, dvt, :], in0=ctxu[:, dvt, :], in1=pr_
                    )

                # -- combined out+proj matmul, fp32 residual --
                h32 = acts.tile([P, NDT, QCW], dt.float32, tag=f"h32{qc}",
                                name=f"h32{qc}")
                hb = acts.tile([P, NDT, QCW], dt.bfloat16, tag=f"hb{qc}",
                               name=f"hb{qc}")
                for mt in range(NDT):
                    ps = psum.tile([P, QCW], dt.float32, tag="mm")
                    for kt in range(NDT):
                        nc.tensor.matmul(
                            ps,
                            lhsT=c_wc[:, kt, mt * P:(mt + 1) * P],
                            rhs=ctxn[:, kt, :],
                            start=(kt == 0),
                            stop=(kt == NDT - 1),
                        )
                    nc.vector.scalar_tensor_tensor(
                        out=h32[:, mt, :], in0=ps, scalar=c_bc[:, mt:mt + 1],
                        in1=c_x32[:, mt, qs], op0=OP.add, op1=OP.add,
                    )
                    nc.gpsimd.tensor_copy(out=hb[:, mt, :], in_=h32[:, mt, :])
                if True:
                    h32s.append(h32); hbs.append(hb)

            last_exp = exp_by[(1, H - 1)][-1]
            # ---- FFN + store, per chunk; gelus after all exps ----
            for qc in range(NQC):
                qs = slice(qc * QCW, (qc + 1) * QCW)
                h32, hb = h32s[qc], hbs[qc]
                g = acts.tile([P, NF1, QCW], dt.bfloat16, tag=f"g{qc}",
                              name=f"g{qc}")
                for mt in range(NF1):
                    ps = psS.tile([P, QCW], dt.float32, tag="score")
                    for kt in range(NDT):
                        nc.tensor.matmul(
                            ps,
                            lhsT=c_wf1[:, kt, mt * P:(mt + 1) * P],
                            rhs=hb[:, kt, :],
                            start=(kt == 0),
                            stop=(kt == NDT - 1),
                        )
                    gi = nc.scalar.activation(
                        out=g[:, mt, :], in_=ps,
                        func=AF.Gelu, bias=c_bf1[:, mt:mt + 1],
                    )
                    add_dep_helper(gi.ins, last_exp, sync=False,
                                   reason="act table: gelu after all exp")
                o32 = acts.tile([P, NDT, QCW], dt.float32, tag=f"o32{qc}",
                                name=f"o32{qc}")
                for mt in range(NDT):
                    ps = psum.tile([P, QCW], dt.float32, tag="mm")
                    for kt in range(NF1):
                        nc.tensor.matmul(
                            ps,
                            lhsT=c_wf2[:, kt, mt * P:(mt + 1) * P],
                            rhs=g[:, kt, :],
                            start=(kt == 0),
                            stop=(kt == NF1 - 1),
                        )
                    nc.vector.scalar_tensor_tensor(
                        out=o32[:, mt, :], in0=ps, scalar=c_bf2[:, mt:mt + 1],
                        in1=h32[:, mt, :], op0=OP.add, op1=OP.add,
                    )

                # -- transpose to token-major, zero padded rows, store --
                for tt in range(qc * NKT // NQC, (qc + 1) * NKT // NQC):
                    to = tt - qc * NKT // NQC
                    ot = outp.tile([P, D], dt.float32, tag="ot")
                    for dtt in range(NDT):
                        pt = psC.tile([P, P], dt.float32, tag="ctx")
                        nc.tensor.transpose(
                            pt, o32[:, dtt, to * P:(to + 1) * P], c_id32
                        )
                        nc.vector.tensor_scalar_mul(
                            out=ot[:, dtt * P:(dtt + 1) * P], in0=pt,
                            scalar1=c_qm[:, tt:tt + 1],
                        )
                    nc.sync.dma_start(out=y[tt * P:(tt + 1) * P, :], in_=ot)

    nc.compile()
    return nc


def _get_module(n_iters: int = 1):
    if n_iters not in _BUILT:
        _BUILT[n_iters] = _build_module(n_iters)
    return _BUILT[n_iters]


def _rearr(a, nt):
    """[nt*128, F] row-major -> device layout [128, nt, F]."""
    f = a.shape[1]
    return np.ascontiguousarray(a.reshape(nt, P, f).transpose(1, 0, 2))


def prepare_in_maps(inputs):
    src = np.asarray(inputs["src"], F32)
    mask = np.asarray(inputs["src_key_padding_mask"])
    in_proj_w = np.asarray(inputs["in_proj_w"], F32)
    in_proj_b = np.asarray(inputs["in_proj_b"], F32)
    out_w = np.asarray(inputs["out_w"], F32)
    out_b = np.asarray(inputs["out_b"], F32)
    proj_w = np.asarray(inputs["proj_w"], F32)
    proj_b = np.asarray(inputs["proj_b"], F32)
    ff1_w = np.asarray(inputs["ff1_w"], F32)
    ff1_b = np.asarray(inputs["ff1_b"], F32)
    ff2_w = np.asarray(inputs["ff2_w"], F32)
    ff2_b = np.asarray(inputs["ff2_b"], F32)

    scale = 1.0 / np.sqrt(F32(DH))
    wq = in_proj_w[:D] * scale
    bq = in_proj_b[:D] * scale
    wk = in_proj_w[D:2 * D]
    bk = in_proj_b[D:2 * D]
    wv_ = in_proj_w[2 * D:]
    bv = in_proj_b[2 * D:]

    wqk_dev = _rearr(np.concatenate([wq, wk], 0).T, NDT).astype(BF16)
    wv_dev = _rearr(wv_.T, NDT).astype(BF16)
    wc_mat = proj_w @ out_w
    wc_dev = _rearr(wc_mat.T, NDT).astype(BF16)
    bo2 = out_b + out_w @ bv
    bc_vec = proj_w @ bo2 + proj_b
    wf1_dev = _rearr(ff1_w.T, NDT).astype(BF16)
    wf2_dev = _rearr(ff2_w.T, NF1).astype(BF16)

    bqk_dev = np.ascontiguousarray(
        np.concatenate([bq, bk]).reshape(4, P).T).astype(F32)
    bc_dev = np.ascontiguousarray(bc_vec.reshape(NDT, P).T).astype(F32)
    bf1_dev = np.ascontiguousarray(ff1_b.reshape(NF1, P).T).astype(F32)
    bf2_dev = np.ascontiguousarray(ff2_b.reshape(NDT, P).T).astype(F32)

    sel_dev = np.zeros((64, P), BF16)
    for j in range(4):
        sel_dev[j, j * DH:(j + 1) * DH] = 1
        sel_dev[32 + j, j * DH:(j + 1) * DH] = 1

    shared = {
        "wqk": wqk_dev, "wv": wv_dev, "wc": wc_dev,
        "wf1": wf1_dev, "wf2": wf2_dev,
        "bqk": bqk_dev, "bc": bc_dev, "bf1": bf1_dev, "bf2": bf2_dev,
        "sel": sel_dev,
    }

    ki = np.arange(L, dtype=F32)[:, None]
    qi = np.arange(L, dtype=F32)[None, :]
    dist = np.abs(qi - ki)

    in_maps = []
    for b in range(NCORES):
        s = int((~mask[b]).sum())
        xT = src[b].T  # [D, L]
        m = (s - dist) / F32(s)
        e = np.exp(m).astype(F32) * (np.arange(L)[:, None] < s)
        # E restricted to diagonal-crossing tiles: tile kt vs q-chunk kt//4
        e_r = _rearr(e.astype(F32), NKT)            # [P, NKT, L]
        ed = np.stack([e_r[:, kt, (kt // 4) * QCW:(kt // 4 + 1) * QCW]
                       for kt in range(NKT)], axis=1)
        kvec = np.arange(L, dtype=np.float64)
        # aux rows (r32, r33, r34) broadcast over (group, head-slot):
        #   k side:  [1, k/s, 1 + pad(k)*(-1e5)]
        #   q side +: [-q/s, +1, +1]   q side -: [+q/s, -1, +1]
        pad_k = (kvec >= s) * (-1e5)
        kaux3 = np.stack([np.ones(L), kvec / s, 1.0 + pad_k], axis=0)
        qp3 = np.stack([-kvec / s, np.ones(L), np.ones(L)], axis=0)
        qm3 = np.stack([kvec / s, -np.ones(L), np.ones(L)], axis=0)

        def _aux(a):
            return np.ascontiguousarray(
                np.broadcast_to(a[None, :, None, :], (2, 3, 4, L))).astype(BF16)

        im = dict(shared)
        im["xtb"] = _rearr(xT, NDT).astype(BF16)
        im["xt32"] = _rearr(xT, NDT).astype(F32)
        im["ed"] = np.ascontiguousarray(ed).astype(BF16)
        im["qauxp"] = _aux(qp3)
        im["qauxm"] = _aux(qm3)
        im["kaux"] = _aux(kaux3)
        im["qm"] = np.ascontiguousarray(
            (np.arange(L) < s).astype(F32).reshape(NKT, P).T)
        in_maps.append(im)
    return in_maps


def run_on_device(inputs, n_iters: int = 1, trace: bool = False):
    from concourse import bass_utils
    nc = _get_module(n_iters)
    in_maps = prepare_in_maps(inputs)
    res = bass_utils.run_bass_kernel_spmd(
        nc, in_maps, core_ids=list(range(NCORES)), trace=trace)
    return res


def kernel(**inputs) -> np.ndarray:
    res = run_on_device(inputs)
    out = np.stack([res.results[b]["y"] for b in range(NCORES)], axis=0)
    return out.astype(F32)
